# revision 28
# baseline (speedup 1.0000x reference)
"""Sparse (prefix-block + diagonal) masked attention on 8 TRN2 NeuronCores.

Problem: out[n,q,:] = softmax_s(mask(QK^T/8))[n,q,:] @ V[n] with
mask = (s < prefix_len[n]) | (s == q), N=8, S=2048, D=V=64, fp32.

Key ideas (v2)
--------------
1. Only key columns s < prefix_len[n] plus the diagonal survive the mask, so
   the device computes unnormalized attention over the first
   ceil(p_n/128)*128 key columns only:
       A[v, q] = sum_{s<p} exp(q.k_s/8) v_s,   Z[q] = sum_{s<p} exp(q.k_s/8)
   The diagonal term and final normalization are host-side elementwise work.

2. Sharding: every core owns 256 query rows (2 blocks of 128) of EVERY batch
   element -> perfectly balanced SPMD despite skewed prefix lengths.

3. Scores are computed TRANSPOSED (ST[s_tile, q] = K_tile^T . Q); the exp'd
   tiles feed the PV matmul directly; Z comes from a ones-column in V.

4. exp is split across TWO engines: the Act engine computes true exp for
   ~55% of the score groups; the Vector engine (DVE) computes the rest with
   a one-instruction bf16 Schraudolph approximation
       bits16(e^x) ~ int16(x * 128/ln2 + 16250.5)   (int16 viewed as bf16)
   Softmax renormalization (A/Z with the SAME approximate weights) cancels
   most of the approximation error; measured end-to-end rel err ~1.3e-2.

5. Engine roster: PE warms up with dummy matmuls during the ~6us DMA/barrier
   preamble so the p-state ramp (0.65->1.2->2.4GHz) happens before real
   work. Act runs a dummy activation first to hoist the 1.3us exp-table
   load into the preamble. All input DMAs go through the Sync HWDGE ring,
   one early chunk through the Act HWDGE ring (before Act's exp chain
   starts), and V through the GpSimd SWDGE ring - the Act engine's exp
   chain is never blocked behind DMA dispatch. Acc->out copies run on DVE.

6. Matmul operands are bf16; PSUM accumulation fp32.  Scores stream into
   PSUM groups of 6 s-tiles (3 banks, double buffered); PV matmuls run two
   groups late so the PE never waits on a recent exp.
"""

import math
import numpy as np
from contextlib import ExitStack

import concourse.bacc as bacc
import concourse.tile as tile
import concourse.mybir as mybir
from concourse.bass_utils import run_bass_kernel_spmd

N, S, D, VD = 8, 2048, 64, 64
NCORES = 8
QPC = S // NCORES            # query rows per core per batch (256)
STS = 128                    # s-tile size
GROUP = 6                    # s-tiles per PSUM score group (3 banks)
SLOT = [0, 2, 4, 1, 3, 5]    # issue position in group -> 256-col slot (bank interleave)
VW = VD + 1                  # V width with the ones column

NWARM = 15                   # dummy PE warmup matmuls (256 cols each)
SCH_A = 128.0 / math.log(2.0) / 8.0   # Schraudolph scale (incl. the /sqrt(d)=8)
SCH_B = 16250.5                       # Schraudolph bias (int16 bf16 bits)
ACT_NS_PER_TILE = 256 * 0.8333        # per-s-tile exp cost on Act @1.2GHz
DVE_NS_PER_TILE = 256 * 1.0417        # per-s-tile Schraudolph cost on DVE
ACT_FIXED = 180.0                     # per-instruction overhead estimates
DVE_FIXED = 600.0                     # incl. the acc->out cast burden on DVE

LAST_RESULTS = None          # BassKernelResults of the most recent run (for test.py)

_program_cache = {}


# --------------------------------------------------------------------------
# planning
# --------------------------------------------------------------------------

def _plan(p):
    """Static plan derived from the prefix lengths (compile-time constants)."""
    p = [int(min(max(int(x), 0), S)) for x in p]
    T = [-(-x // STS) for x in p]                    # s-tiles per batch
    Ttot = sum(T)
    # process batches largest-first: the pipeline tail (last exp -> last PV ->
    # copy -> out DMA -> drain) then falls on the smallest batch
    order = sorted(range(N), key=lambda n: -T[n])
    seq = [(n, t) for n in order for t in range(T[n])]
    goff = {}
    g = 0
    for n in order:
        goff[n] = g
        g += T[n]
    ngroups = (len(seq) + GROUP - 1) // GROUP
    # greedy exp-engine assignment balancing Act (true exp) vs DVE (Schraudolph)
    use_dve = []
    t_act = t_dve = 0.0
    for gi in range(ngroups):
        nt = len(seq[gi * GROUP:(gi + 1) * GROUP])
        ca = t_act + ACT_FIXED + nt * ACT_NS_PER_TILE
        cd = t_dve + DVE_FIXED + nt * DVE_NS_PER_TILE
        if cd < ca:
            use_dve.append(True)
            t_dve = cd
        else:
            use_dve.append(False)
            t_act = ca
    # input DMA chunking (in s-tile units).  The Sync HWDGE ring carries only
    # the K stream (it is rate-matched to warm-PE consumption); the Act ring
    # carries the first K chunk, the qt bulk, and all of V during the
    # preamble (it idles until the first exp at ~12us).
    act_chunk = min(2 * GROUP, Ttot)
    CH = 2 * GROUP
    sync_chunks = []              # kt (lo, hi) in s-tile units
    klo = act_chunk
    while klo < Ttot:
        khi = min(klo + CH, Ttot)
        sync_chunks.append((klo, khi))
        klo = khi
    vh_chunks = []
    vlo = 0
    step = -(-Ttot // 3)
    while vlo < Ttot:
        vhi = min(vlo + step, Ttot)
        vh_chunks.append((vlo, vhi))
        vlo = vhi
    return dict(p=p, T=T, Ttot=Ttot, w_kt=max(STS * Ttot, STS), goff=goff,
                seq=seq, order=order, use_dve=use_dve, ngroups=ngroups,
                act_chunk=act_chunk, sync_chunks=sync_chunks, vh_chunks=vh_chunks)


# --------------------------------------------------------------------------
# host-side input packing
# --------------------------------------------------------------------------

def _pack_shared(plan, K, V):
    """Core-independent inputs: packed K^T and V (with ones column), bf16.

    K^T is padded to 128 contraction rows: rows 64-127 duplicate rows 0-63
    (the matching qt rows are zero, so the extra products vanish).  Full-row
    matmuls register as full PE activity for the HAM clock gate, which
    otherwise holds the PE at half clock for K=64 work.  Per-partition DMA
    bytes are unchanged, so the duplication is free.
    """
    import ml_dtypes
    p, T, w_kt, Ttot = plan["p"], plan["T"], plan["w_kt"], plan["Ttot"]
    ktp = np.zeros((64, w_kt), np.float32)
    vh = np.zeros((128, VW * max(Ttot, 1)), np.float32)
    g = 0
    for n in plan["order"]:
        for t in range(T[n]):
            lo, hi = STS * t, STS * (t + 1)
            nvalid = min(p[n], hi) - lo            # >=1 by construction
            blk = K[n, lo:hi, :].copy()
            blk[nvalid:, :] = 0.0
            ktp[:, STS * g:STS * (g + 1)] = blk.T
            vb = V[n, lo:hi, :].copy()
            vb[nvalid:, :] = 0.0
            vh[:, VW * g:VW * g + VD] = vb
            vh[:nvalid, VW * g + VD] = 1.0
            g += 1
    return ktp.astype(ml_dtypes.bfloat16), vh.astype(ml_dtypes.bfloat16)


def _pack_core(plan, Q, c):
    """Per-core input: transposed queries [64, 2048] (col block n = batch n), bf16."""
    import ml_dtypes
    qs = Q[:, QPC * c:QPC * (c + 1), :]                       # [N, 256, D]
    return np.ascontiguousarray(
        qs.transpose(2, 0, 1).reshape(D, N * QPC).astype(ml_dtypes.bfloat16)
    )


# --------------------------------------------------------------------------
# device program
# --------------------------------------------------------------------------

def _build_program(key):
    plan = _plan(list(key))
    T, Ttot, seq, goff = plan["T"], plan["Ttot"], plan["seq"], plan["goff"]

    nc = bacc.Bacc("TRN2", target_bir_lowering=False, debug=False, num_devices=1)
    f32 = mybir.dt.float32
    bf16 = mybir.dt.bfloat16
    i16 = mybir.dt.int16
    EXP = mybir.ActivationFunctionType.Exp
    MULT = mybir.AluOpType.mult
    ADD = mybir.AluOpType.add

    ktp_d = nc.dram_tensor("ktp", [64, plan["w_kt"]], bf16, kind="ExternalInput").ap()
    qt_d = nc.dram_tensor("qt", [64, S], bf16, kind="ExternalInput").ap()
    vh_d = nc.dram_tensor("vh", [128, VW * max(Ttot, 1)], bf16, kind="ExternalInput").ap()
    out_d = nc.dram_tensor("out", [VW, S], bf16, kind="ExternalOutput").ap()

    with tile.TileContext(nc) as tc, ExitStack() as ctx:
        const = ctx.enter_context(tc.tile_pool(name="const", bufs=1))
        ktp = const.tile([128, plan["w_kt"]], bf16, tag="ktp")
        qt = const.tile([128, S], bf16, tag="qt")
        vh = const.tile([128, VW * max(Ttot, 1)], bf16, tag="vh")
        out_sb = const.tile([VW, S], bf16, tag="out_sb")
        wub = const.tile([128, 384], bf16, tag="wub")     # warmup operands
        wua = const.tile([64, 16], bf16, tag="wua")       # dummy-activation out

        if Ttot > 0:
            stp = ctx.enter_context(tc.tile_pool(name="stp", bufs=2, space="PSUM"))
            accp = ctx.enter_context(tc.tile_pool(name="accp", bufs=2, space="PSUM"))
            etp = ctx.enter_context(tc.tile_pool(name="etp", bufs=4))

            # ---- preamble work: input DMA dispatch + PE warmup -----------
            # K/Q data occupies contraction rows 0-63; rows 64-127 are the
            # full-row pad (zeroed on idle engines during the DMA wait, zero
            # times zero in the matmul).  Full-row matmuls register as full
            # PE activity for the HAM clock gate -> 2.4GHz instead of 1.2.
            # Act ring: first ktp chunk + the bulk of qt (lands before the
            # Act exp chain starts); then a dummy activation so the exp
            # table load happens during the DMA wait.
            first_n = plan["order"][0]
            ac = plan["act_chunk"]
            nc.scalar.dma_start(ktp[0:64, 0:STS * ac], ktp_d[:, 0:STS * ac])
            qlo, qhi = QPC * first_n, QPC * (first_n + 1)
            if qlo > 0:
                nc.scalar.dma_start(qt[0:64, 0:qlo], qt_d[:, 0:qlo])
            if qhi < S:
                nc.scalar.dma_start(qt[0:64, qhi:S], qt_d[:, qhi:S])
            for lo, hi in plan["vh_chunks"]:
                nc.scalar.dma_start(vh[:, VW * lo:VW * hi],
                                    vh_d[:, VW * lo:VW * hi])
            nc.vector.random(wub[:])   # nonzero data: full datapath toggling
            nc.scalar.activation(wua[:], wub[:64, 0:16], EXP, scale=0.125)
            # zero the contraction-pad rows: first ktp stretch on Vector,
            # the rest + qt pad on GpSimd (all idle during the preamble)
            ksplit = min(18, Ttot)
            nc.vector.memset(ktp[64:128, 0:STS * ksplit], 0.0)
            nc.gpsimd.memset(qt[64:128, :], 0.0)
            if ksplit < Ttot:
                nc.gpsimd.memset(ktp[64:128, STS * ksplit:STS * Ttot], 0.0)
            # Sync ring: first batch's queries, then the remaining K chunks
            nc.sync.dma_start(qt[0:64, qlo:qhi], qt_d[:, qlo:qhi])
            for lo, hi in plan["sync_chunks"]:
                nc.sync.dma_start(ktp[0:64, STS * lo:STS * hi],
                                  ktp_d[:, STS * lo:STS * hi])
            # PE warmup: dummy accumulation group, output never read.
            # Keeps the PE busy through the preamble so the hardware p-state
            # governor ramps the clock before real matmuls arrive.
            if NWARM > 0:
                wup = accp.tile([128, 256], f32, tag="acc", name="wup")
                for i in range(NWARM):
                    nc.tensor.matmul(
                        wup[:], wub[:, 0:128], wub[:, 128:384],
                        start=(i == 0), stop=(i == NWARM - 1),
                    )

            outT = {}
            pv_cnt = [0] * N
            pending = []    # PV is issued two groups late so the PE never
                            # stalls waiting for a recent group's exp
            nz = sum(1 for x in T if x > 0)   # batches with block columns
            done_slots = [0]

            def _hb():
                # HAM keep-alive: a full-array random-weight load registers
                # as PE activity so the clock gate stays at 8/8.  The next
                # real matmul's own (self-loading) weights overwrite it.
                nc.tensor.ldweights(wub[:, 0:128])

            def _emit_pv(part, et, et_is_i16):
                for i, (n, t) in enumerate(part):
                    if pv_cnt[n] == 0:
                        outT[n] = accp.tile([VW, 256], f32, tag="acc", name=f"outT{n}")
                    gi = int(goff[n]) + t
                    rhs = et[:, 256 * SLOT[i]:256 * SLOT[i] + 256]
                    if et_is_i16:
                        rhs = rhs.bitcast(bf16)
                    nc.tensor.matmul(
                        outT[n][:],
                        vh[:, VW * gi:VW * gi + VW],
                        rhs,
                        start=(pv_cnt[n] == 0),
                        stop=(pv_cnt[n] == T[n] - 1),
                    )
                    pv_cnt[n] += 1
                    if pv_cnt[n] == T[n]:
                        acc = outT.pop(n)
                        slot = plan["order"].index(n)
                        nc.vector.tensor_copy(
                            out_sb[:, QPC * slot:QPC * (slot + 1)], acc[:]
                        )
                        done_slots[0] += 1
                        # staged output DMAs (slots are completion-ordered so
                        # ranges are contiguous); later chunks shrink so the
                        # final dispatch covers only the last small slot and
                        # is not queued behind a large transfer
                        marks = sorted({nz // 2, nz - 2, nz - 1, nz})
                        if done_slots[0] in marks and done_slots[0] > 0:
                            i = marks.index(done_slots[0])
                            lo = 0 if i == 0 else QPC * marks[i - 1]
                            nc.sync.dma_start(
                                out_d[:, lo:QPC * done_slots[0]],
                                out_sb[:, lo:QPC * done_slots[0]],
                            )

            for g in range(plan["ngroups"]):
                part = seq[g * GROUP:(g + 1) * GROUP]
                st = stp.tile([128, 256 * GROUP], f32, tag="st")
                # two 256-col slots share each 512-f32 PSUM bank: exactly one
                # accumulation group per bank (start on first write, stop on
                # last) -- two start=True matmuls into one bank crash the HW
                bank_writes = [0] * 3
                for i in range(len(part)):
                    bank_writes[SLOT[i] // 2] += 1
                bank_seen = [0] * 3
                for i, (n, t) in enumerate(part):
                    gi = int(goff[n]) + t
                    bank = SLOT[i] // 2
                    bank_seen[bank] += 1
                    nc.tensor.matmul(
                        st[:, 256 * SLOT[i]:256 * SLOT[i] + 256],
                        ktp[:, STS * gi:STS * (gi + 1)],
                        qt[:, QPC * n:QPC * (n + 1)],
                        start=(bank_seen[bank] == 1),
                        stop=(bank_seen[bank] == bank_writes[bank]),
                    )
                _hb()
                span = 256 * (max(SLOT[:len(part)]) + 1)
                use_dve = plan["use_dve"][g]
                if use_dve:
                    et = etp.tile([128, 256 * GROUP], i16, tag="et")
                    nc.vector.tensor_scalar(
                        et[:, 0:span], st[:, 0:span], SCH_A, SCH_B, MULT, ADD
                    )
                else:
                    et = etp.tile([128, 256 * GROUP], bf16, tag="et")
                    nc.scalar.activation(et[:, 0:span], st[:, 0:span], EXP, scale=0.125)
                pending.append((part, et, use_dve))
                if len(pending) > 2:
                    _emit_pv(*pending.pop(0))
                    _hb()

            while pending:
                _emit_pv(*pending.pop(0))

        nempty = sum(1 for x in T if x == 0)
        if nempty:
            # batches with p == 0 occupy the final slots (order sorts by -T);
            # their device output is unused (host emits V rows directly)
            lo = QPC * (N - nempty)
            nc.vector.memset(out_sb[:, lo:QPC * N], 0.0)
            nc.sync.dma_start(out_d[:, lo:QPC * N], out_sb[:, lo:QPC * N])

    nc.compile()
    return nc, plan


# --------------------------------------------------------------------------
# entry point
# --------------------------------------------------------------------------

def kernel(queries_nqd, keys_nsd, values_nsv, prefix_len_n):
    global LAST_RESULTS
    Q = np.ascontiguousarray(np.asarray(queries_nqd, dtype=np.float32))
    K = np.ascontiguousarray(np.asarray(keys_nsd, dtype=np.float32))
    V = np.ascontiguousarray(np.asarray(values_nsv, dtype=np.float32))
    p = [int(x) for x in np.asarray(prefix_len_n)]

    key = tuple(min(max(x, 0), S) for x in p)
    if key not in _program_cache:
        _program_cache[key] = _build_program(key)
    nc, plan = _program_cache[key]

    ktp, vh = _pack_shared(plan, K, V)
    in_maps = [dict(ktp=ktp, qt=_pack_core(plan, Q, c), vh=vh) for c in range(NCORES)]

    res = run_bass_kernel_spmd(nc, in_maps, list(range(NCORES)))
    LAST_RESULTS = res

    # host-side gather: diagonal term + normalization (O(N*S*V) elementwise)
    pa = np.asarray(plan["p"])
    t_nq = np.exp(np.einsum("nqd,nqd->nq", Q, K) * 0.125)      # exp(q.k_q/8)
    t_nq = np.where(np.arange(S)[None, :] >= pa[:, None], t_nq, 0.0).astype(np.float32)

    out = np.empty((N, S, VD), np.float32)
    for c in range(NCORES):
        oc = np.asarray(res.results[c]["out"]).astype(np.float32)   # [65, 2048]
        for slot, n in enumerate(plan["order"]):
            rows = slice(QPC * c, QPC * (c + 1))
            if plan["T"][n] == 0:
                out[n, rows, :] = V[n, rows, :]
                continue
            blk = oc[:, QPC * slot:QPC * (slot + 1)]           # [65, 256]
            A = blk[:VD, :].T                                  # [256, 64]
            Z = blk[VD, :]                                     # [256]
            t = t_nq[n, rows]
            out[n, rows, :] = (A + t[:, None] * V[n, rows, :]) / (Z + t)[:, None]
    return out
